# revision 74
# baseline (speedup 1.0000x reference)
"""Sparse (class-gated bilinear) attention kernel for TRN2, 8 NeuronCores.

Problem shapes (hardcoded): b=2, h=8, s=512, d=64, C=8 classes, B=4 bases.

Math (per b,h):
  W1e[c] = (sum_B softmax(alpha1)[c,B,h] * W1[B,h]) / sqrt(d)   (host)
  W2e[c] = sum_B softmax(alpha2)[c,B,h] * W2[B,h]               (host)
  UT_c[n,i] = sum_m W1e[c][m,n] * Q[i,m]                        (PE, packed pairs)
  S_c[j,i]  = sum_n K[j,n] * UT_c[n,i]                          (PE, f32r)
  x_c = exp(S_c)                 (ACT, batched 2 classes/op from PSUM)
  R_c[j,i] = exp(rpb[i,j]) * [bmat[i,j]==c]                     (host, bf16)
  ec_c = x_c . R_c               (DVE, one [128,4096] mul per step)
  t_c[j,D] = sum_d V[j,d] W2e[c][d,D]; taug = [t_c | ones]      (PE + copy)
  out[D,i] (D<64) = sum_c sum_j taug_c[j,D] ec_c[j,i]           (PE, accum)
  out[64,i]       = sum_c sum_j ec_c[j,i] = Z[i]                (same matmuls)
  final out[i,D] = out[D,i] / Z[i]                              (host)

Sharding: 16 (b,h) pairs over 8 cores; core k handles b=k//4,
heads (2*(k%4), 2*(k%4)+1).
"""

import os
import sys

import numpy as np

if "/opt/trn_rl_repo" not in sys.path:
    sys.path.insert(0, "/opt/trn_rl_repo")

import ml_dtypes

B_, H_, S_, D_, C_ = 2, 8, 512, 64, 8
NCORES = 8
JT = S_ // 128  # 4 j-tiles
NSTEP = 2 * JT  # (jt, p) steps

_CACHE = {}


def _softmax(a, axis):
    e = np.exp(a - a.max(axis=axis, keepdims=True))
    return e / e.sum(axis=axis, keepdims=True)


def _build_nc():
    import concourse.bass as bass  # noqa: F401
    import concourse.mybir as mybir
    from concourse import bacc
    from concourse.tile import TileContext

    f32 = mybir.dt.float32
    f32r = mybir.dt.float32r
    bf16 = mybir.dt.bfloat16

    nc = bacc.Bacc("TRN2", target_bir_lowering=False, debug=False)

    # packed inputs: inp = (w1 | qt | kt), vw = (vt | w2) — fewer DMAs,
    # the HWDGE dispatch chain is near-critical at the head
    inp_d = nc.dram_tensor("inp", [2, 64, 1536], f32r, kind="ExternalInput").ap()
    vw_d = nc.dram_tensor("vw", [2, 64, 1024], f32r, kind="ExternalInput").ap()
    rr_d = nc.dram_tensor("rr", [NSTEP, 128, 4096], bf16, kind="ExternalInput").ap()
    ot_d = nc.dram_tensor("ot", [2, 65, 512], f32, kind="ExternalOutput").ap()

    EXP = mybir.ActivationFunctionType.Exp

    with TileContext(nc) as tc:
        with (
            tc.tile_pool(name="inp", bufs=1) as ipool,
            tc.tile_pool(name="work", bufs=2) as wpool,
            tc.tile_pool(name="pst", bufs=3, space="PSUM") as pst,
            tc.tile_pool(name="pacc", bufs=1, space="PSUM") as pacc,
        ):
            # DMA order: packed inputs first, then column-halved rr so each
            # ec half-multiply can start as soon as its half lands (the
            # serial HWDGE dispatch chain is near-critical at the head).
            rr = [None] * NSTEP

            def dma_rr_half(s, h):
                sl = slice(h * 2048, (h + 1) * 2048)
                nc.sync.dma_start(out=rr[s][:, sl], in_=rr_d[s][:, sl])

            for s in range(NSTEP):
                rt_ = ipool.tile([128, 4096], bf16, tag=f"rr{s}", name=f"rr{s}")
                rr[s] = rt_
            inp, kt2, vw = {}, {}, {}
            w1, qt, kt, vt, w2 = {}, {}, {}, {}, {}
            for p in range(2):
                inp[p] = ipool.tile(
                    [64, 1536], f32r, tag=f"inp{p}", name=f"inp{p}"
                )
                w1[p] = inp[p][:, 0:512]
                qt[p] = inp[p][:, 512:1024]
                # kt duplicated into partitions 64-127 (kt2) so ST matmuls
                # can use partition-offset ut slices (lhsT/rhs base match)
                kt2[p] = ipool.tile(
                    [128, 512], f32r, tag=f"kt2{p}", name=f"kt2{p}"
                )
                kt[p] = inp[p][:, 1024:1536]
                vw[p] = ipool.tile(
                    [64, 1024], f32r, tag=f"vw{p}", name=f"vw{p}"
                )
                vt[p] = vw[p][:, 0:512]
                w2[p] = vw[p][:, 512:1024]
            for p in range(2):
                nc.sync.dma_start(out=inp[p], in_=inp_d[p])
                nc.sync.dma_start(
                    out=kt2[p][64:128, :], in_=inp_d[p][:, 1024:1536]
                )
            nc.sync.dma_start(out=vw[0], in_=vw_d[0])
            nc.sync.dma_start(out=vw[1], in_=vw_d[1])
            for s in range(NSTEP):
                dma_rr_half(s, 0)
                dma_rr_half(s, 1)

            us, tsb = {0: [], 1: []}, {0: [], 1: []}
            ot_ps = {}
            for p in range(2):
                # [128,512] so the full bank can serve as head-phase matmul
                # scratch; accumulation only uses rows [:65].
                ot_ps[p] = pacc.tile(
                    [128, 512], f32, tag=f"o{p}", name=f"ot{p}"
                )

            def ut_pair(p, g, up):
                # UT pair g -> [128,512] PSUM (rows 0-63 = class 2g,
                # 64-127 = 2g+1), copied to f32r SBUF; ST rhs uses
                # partition slices of the copy.
                nc.tensor.matmul(
                    up, w1[p][:, g * 128 : (g + 1) * 128], qt[p],
                    start=True, stop=True,
                )
                uc = ipool.tile(
                    [128, 512], f32r, tag=f"ut{p}_{g}", name=f"ut{p}_{g}"
                )
                nc.vector.tensor_copy(out=uc, in_=up)
                for h_ in range(2):
                    us[p].append(uc[h_ * 64 : (h_ + 1) * 64, :])

            def ut_mm2(p, gg, split=False):
                # Two UT pair matmuls share one ring tile; copied out merged
                # (one DVE op) or per-pair (split=True: first ST batch can
                # start after just the first half lands).
                up = pst.tile([128, 1024], f32, tag="st")
                for i, g in enumerate((2 * gg, 2 * gg + 1)):
                    nc.tensor.matmul(
                        up[:, i * 512 : (i + 1) * 512],
                        w1[p][:, g * 128 : (g + 1) * 128], qt[p],
                        start=True, stop=True,
                    )
                uc = ipool.tile(
                    [128, 1024], f32r, tag=f"ut{p}_p{gg}", name=f"ut{p}_p{gg}"
                )
                if split == "act":
                    nc.scalar.copy(uc, up)
                elif split:
                    nc.vector.tensor_copy(out=uc[:, :512], in_=up[:, :512])
                    nc.vector.tensor_copy(out=uc[:, 512:], in_=up[:, 512:])
                else:
                    nc.vector.tensor_copy(out=uc, in_=up)
                for i in range(2):
                    for h_ in range(2):
                        us[p].append(
                            uc[h_ * 64 : (h_ + 1) * 64,
                               i * 512 : (i + 1) * 512]
                        )

            def emit_tsb(p, jt, tp=None):
                # taug[j, (c,65)] = [t_c | ones]
                if tp is None:
                    tp = pst.tile([128, 1024], f32, tag="st")
                nc.tensor.matmul(
                    tp[:, :512], vt[p][:, jt * 128 : (jt + 1) * 128], w2[p],
                    start=True, stop=True,
                )
                ts = ipool.tile(
                    [128, 520], bf16, tag=f"t{p}_{jt}", name=f"t{p}_{jt}"
                )
                tsv = ts.rearrange("p (c x) -> p c x", c=C_)
                nc.gpsimd.memset(tsv[:, :, 64:65], 1.0)
                nc.vector.tensor_copy(
                    out=tsv[:, :, :64],
                    in_=tp[:, :512].rearrange("p (c x) -> p c x", c=C_),
                )
                tsb[p].append(ts)

            # Steps: 8 ST matmuls -> 4 batched exps -> ec mul in halves.
            # Output matmuls for step s are emitted during step s+1.
            pending = None

            def flush_pending(cs=range(C_)):
                ec_, p_, jt_ = pending
                for c in cs:
                    nc.tensor.matmul(
                        ot_ps[p_][:65],
                        tsb[p_][jt_][:, c * 65 : (c + 1) * 65],
                        ec_[:, c * 512 : (c + 1) * 512],
                        start=(jt_ == 0 and c == 0),
                        stop=(jt_ == JT - 1 and c == C_ - 1),
                        skip_group_check=True,
                    )

            def st_batch(p, jt, e, xall):
                rt = pst.tile([128, 1024], f32, tag="st")
                for h_ in range(2):
                    c = 2 * e + h_
                    ktv = (
                        kt[p][:, jt * 128 : (jt + 1) * 128]
                        if c % 2 == 0
                        else kt2[p][64:128, jt * 128 : (jt + 1) * 128]
                    )
                    nc.tensor.matmul(
                        rt[:, h_ * 512 : (h_ + 1) * 512],
                        ktv, us[p][c],
                        start=True, stop=True,
                    )
                nc.scalar.activation(
                    xall[:, e * 1024 : (e + 1) * 1024], rt, EXP
                )

            def ec_half(s, xall, ec, h):
                half = slice(h * 2048, (h + 1) * 2048)
                nc.vector.tensor_mul(ec[:, half], xall[:, half], rr[s][:, half])

            def emit_step(p, jt):
                # DVE queue order per step: [tsb copy, ec halves] so the tsb
                # tile is ready before flush(s) runs during step s+1.
                nonlocal pending
                s = 2 * jt + p
                xall = wpool.tile([128, 4096], bf16, tag="xall")
                ec = wpool.tile([128, 4096], bf16, tag="ec")
                for e in range(4):
                    st_batch(p, jt, e, xall)
                if pending is not None:
                    flush_pending()
                emit_tsb(p, jt)
                if (p, jt) == (0, JT - 1):
                    emit_tsb(1, JT - 1)  # lookahead for the tail step
                ec_half(s, xall, ec, 0)
                ec_half(s, xall, ec, 1)
                pending = (ec, p, jt)

            # PE warmup: ~3us of dummy matmuls ramps the PE clock to full
            # speed (2.4GHz) before the first real matmul arrives.
            wu = wpool.tile([128, 128], bf16, tag="wu", name="wu", bufs=1)
            nc.vector.memset(wu, 0.0)
            wup = pst.tile([128, 1024], f32, tag="st")
            for _ in range(24):
                nc.tensor.matmul(
                    wup[:1, :128], wu[:, :1], wu, start=True, stop=True,
                )

            # Head: interleave UT pair matmuls+copies into step (0,0)'s
            # batches so ACT (the bottleneck) starts exponentiating as
            # early as possible. Head-1 UT matmuls use the accumulator
            # banks as scratch so the 3-slot ring stays gapless.
            x00 = wpool.tile([128, 4096], bf16, tag="xall", name="x00")
            e00 = wpool.tile([128, 4096], bf16, tag="ec", name="e00")
            ut_mm2(0, 0)
            ut_mm2(0, 1)
            st_batch(0, 0, 0, x00)
            st_batch(0, 0, 1, x00)
            ut_pair(1, 0, ot_ps[0])
            ut_pair(1, 1, ot_ps[1])
            st_batch(0, 0, 2, x00)
            st_batch(0, 0, 3, x00)
            ut_pair(1, 2, ot_ps[0])
            ut_pair(1, 3, ot_ps[1])
            emit_tsb(0, 0)
            ec_half(0, x00, e00, 0)
            ec_half(0, x00, e00, 1)
            pending = (e00, 0, 0)

            emit_step(1, 0)
            for jt_ in range(1, JT):
                emit_step(0, jt_)
                if jt_ < JT - 1:
                    emit_step(1, jt_)

            # Tail: last step (1,3); its tsb was emitted during (0,3) by
            # emit_step's lookahead. ST batches run uninterrupted; ec in
            # quarters so the final out matmuls chase individual exps.
            os_ = {}
            xl = wpool.tile([128, 4096], bf16, tag="xall", name="xl")
            el = wpool.tile([128, 4096], bf16, tag="ec", name="el")

            def ec_q(e):
                q = slice(e * 1024, (e + 1) * 1024)
                nc.vector.tensor_mul(el[:, q], xl[:, q], rr[7][:, q])

            st_batch(1, 3, 0, xl)
            st_batch(1, 3, 1, xl)
            ec_q(0)
            ec_q(1)
            st_batch(1, 3, 2, xl)
            st_batch(1, 3, 3, xl)
            ec_q(2)
            flush_pending()  # outs of (0,3): head 0 accumulation complete
            os_[0] = wpool.tile([65, 512], f32, tag="os", name="os0")
            nc.vector.tensor_copy(out=os_[0], in_=ot_ps[0][:65])
            nc.sync.dma_start(out=ot_d[0], in_=os_[0])
            pending = (el, 1, 3)
            flush_pending(range(6))
            ec_q(3)
            flush_pending(range(6, C_))
            os_[1] = wpool.tile([65, 512], f32, tag="os", name="os1")
            nc.vector.tensor_copy(out=os_[1], in_=ot_ps[1][:65])
            nc.sync.dma_start(out=ot_d[1], in_=os_[1])

    nc.compile()
    return nc


def _get_nc():
    if "nc" not in _CACHE:
        _CACHE["nc"] = _build_nc()
    return _CACHE["nc"]


def kernel(**inputs):
    q = np.asarray(inputs["query"], np.float32)
    k = np.asarray(inputs["key"], np.float32)
    v = np.asarray(inputs["value"], np.float32)
    bm = np.asarray(inputs["b_mat"])
    rpb = np.asarray(inputs["rpb"], np.float32)
    W1 = np.asarray(inputs["W1"], np.float32)
    a1 = np.asarray(inputs["alpha1"], np.float32)
    W2 = np.asarray(inputs["W2"], np.float32)
    a2 = np.asarray(inputs["alpha2"], np.float32)
    mask = np.asarray(inputs["mask"])

    W1e = np.einsum("Bhmn,CBh->Chmn", W1, _softmax(a1, 1)) / np.sqrt(D_)
    W2e = np.einsum("BhdD,CBh->ChdD", W2, _softmax(a2, 1))

    # additive -inf pair mask would go here; spec guarantees mask == ones
    assert mask.all(), "kernel assumes all-ones mask (spec fill=ones)"

    cls = np.arange(C_, dtype=bm.dtype)
    in_maps = []
    for cid in range(NCORES):
        b = cid // 4
        hs = [2 * (cid % 4), 2 * (cid % 4) + 1]
        qt = np.stack([q[b, h].T for h in hs]).astype(np.float32)
        kt = np.stack([k[b, h].T for h in hs]).astype(np.float32)
        vt = np.stack([v[b, h].T for h in hs]).astype(np.float32)
        # [m, C, n] -> [64, 512] per head (class-major columns)
        w1 = np.stack(
            [W1e[:, h].transpose(1, 0, 2).reshape(64, 512) for h in hs]
        ).astype(np.float32)
        w2 = np.stack(
            [W2e[:, h].transpose(1, 0, 2).reshape(64, 512) for h in hs]
        ).astype(np.float32)
        inp = np.concatenate([w1, qt, kt], axis=-1)  # [2, 64, 1536]
        vw = np.concatenate([vt, w2], axis=-1)  # [2, 64, 1024]

        # R[s=2*jt+p][j, c*512+i] = exp(rpb[b,h][i, jt*128+j]) * (bmT==c)
        bmT = bm[b].T  # [j, i]
        onehot = (bmT[:, :, None] == cls).astype(np.float32)  # [j, i, c]
        rr = np.empty((NSTEP, 128, 4096), dtype=ml_dtypes.bfloat16)
        for p, h in enumerate(hs):
            erpT = np.exp(rpb[b, h].T)  # [j, i]
            rf = erpT[:, :, None] * onehot  # [j, i, c]
            rf = rf.transpose(0, 2, 1).reshape(512, 4096)  # [j, (c,i)]
            for jt in range(JT):
                rr[2 * jt + p] = rf[jt * 128 : (jt + 1) * 128].astype(
                    ml_dtypes.bfloat16
                )
        in_maps.append(
            {"inp": inp, "vw": vw, "rr": rr}
        )

    import time

    from concourse.bass_utils import run_bass_kernel_spmd

    try:
        res = run_bass_kernel_spmd(
            _get_nc(), in_maps, core_ids=list(range(NCORES))
        )
    except Exception:
        # transient NRT_EXEC_UNIT_UNRECOVERABLE from a previously wedged
        # device clears on redispatch
        time.sleep(5)
        res = run_bass_kernel_spmd(
            _get_nc(), in_maps, core_ids=list(range(NCORES))
        )
    _CACHE["last_res"] = res
    outs = res.results

    out = np.zeros((B_, H_, S_, D_), np.float32)
    for cid in range(NCORES):
        b = cid // 4
        hs = [2 * (cid % 4), 2 * (cid % 4) + 1]
        for p, h in enumerate(hs):
            ot = np.asarray(outs[cid]["ot"][p], np.float32)  # [65, 512]
            out[b, h] = (ot[:64] / ot[64:65]).T
    return out
